# revision 18
# baseline (speedup 1.0000x reference)
"""Tensor-parallel causal multi-head attention for 8 TRN2 NeuronCores.

Problem: B=2, T=2048, HIDDEN=2048, 16 heads x 128 head_dim, causal, RoPE.
Sharding: 2 heads per core (tensor parallel). Each core computes its QKV
projections, RoPE, causal attention, and a partial output projection over
its 256 hidden features; the host sums the 8 partial outputs.

v2 layout/engine changes vs v1:
  - V computed directly in [t, d] via xc-stationary matmuls (no V^T pass,
    no PE transposes or per-block copies for V).
  - RoPE entirely in bf16 with a host-side half-major head-dim permutation
    (even dims then odd dims) so every DVE operand is packed 16-bit.
  - Q^T/K^T transpose-block copies on DVE; PSUM->SBUF staging split between
    Act and DVE to balance engine load.
  - Causal trim: diagonal k-tiles compute scores/exp/PV only on the valid
    query range (memset of the invalid ptile region keeps masks NaN-safe).

Layouts (per core):
  xt    [2048 c, 4096 t]  bf16   (x transposed; contraction dim on partitions)
  wqt/wkt [2048 c, 256 d] bf16   (head-slice, transposed, half-major permuted)
  wvt   [2048 c, 256 d]  bf16   (head-slice of wv, transposed, unpermuted)
  wot   [256 c, 2048 d]  bf16   (per-core row-slice of wo.T)
  cos2/sin2 [2048 t, 128] bf16  (freqs duplicated across the 2 local heads)
  out   [4096 t, 2048 d]  bf16  partial output (host sums over cores)
"""

import numpy as np
import ml_dtypes
from contextlib import ExitStack

import concourse.bass as bass
import concourse.mybir as mybir
import concourse.tile as tile
from concourse import bacc
from concourse.bass_utils import run_bass_kernel_spmd
from concourse.masks import make_identity

F32 = mybir.dt.float32
BF16 = mybir.dt.bfloat16
FP8E3 = mybir.dt.float8e3

NCORES = 8
B, T, C = 2, 2048, 2048
TT = B * T              # 4096 flattened rows
NH, D = 16, 128         # global heads, head dim
HL = NH // NCORES       # 2 local heads
DH = HL * D             # 256 local head features
NE = 8                  # t-eighths of 512 rows
ET = TT // NE           # 512 rows per eighth
CT = C // 128           # 16 contraction tiles
SCALE = 1.0 / float(np.sqrt(D))

_CACHE: dict = {}
XT_NP_DT = ml_dtypes.bfloat16


def _build(T=T, B=B, num_devices=NCORES, repeat=1, small_out=False,
           ablate=()):
    TT = B * T
    NE = TT // 512
    ET = 512
    nc = bacc.Bacc("TRN2", target_bir_lowering=False, debug=False,
                   num_devices=num_devices)
    xt = nc.dram_tensor("xt", [C, TT], BF16, kind="ExternalInput").ap()
    wqt = nc.dram_tensor("wqt", [C, DH], BF16, kind="ExternalInput").ap()
    wkt = nc.dram_tensor("wkt", [C, DH], BF16, kind="ExternalInput").ap()
    wvt = nc.dram_tensor("wvt", [C, DH], BF16, kind="ExternalInput").ap()
    wot = nc.dram_tensor("wot", [DH, C], BF16, kind="ExternalInput").ap()
    cos2 = nc.dram_tensor("cos2", [T, 2 * (D // 2)], BF16, kind="ExternalInput").ap()
    sin2 = nc.dram_tensor("sin2", [T, 2 * (D // 2)], BF16, kind="ExternalInput").ap()
    _odt = F32 if "f32out" in ablate else BF16
    out = nc.dram_tensor("out", [128 if small_out else TT, C], _odt,
                         kind="ExternalOutput").ap()

    with ExitStack() as ctx:
        tc = ctx.enter_context(tile.TileContext(nc))
        # ---- persistent tiles -------------------------------------------
        gp = ctx.enter_context(tc.tile_pool(name="glob", bufs=1))
        # wqk packs [wq_c | wk_c] per c-tile so Q and K come from ONE
        # N=512 matmul (one PSUM accumulation group per bank).
        wqk_sb = gp.tile([128, CT * 2 * DH], BF16)   # [128, 8192]
        wv_sb = gp.tile([128, CT * DH], BF16)
        wo_sb = gp.tile([128, HL * C], BF16)    # [128, 4096]
        qk_view = wqk_sb[:].rearrange("p (k d) -> p k d", d=2 * DH)
        nc.sync.dma_start(qk_view[:, :, 0:DH],
                          wqt.rearrange("(k p) d -> p k d", p=128))
        nc.sync.dma_start(qk_view[:, :, DH:2 * DH],
                          wkt.rearrange("(k p) d -> p k d", p=128))
        for dst, src_ap, nd in ((wv_sb, wvt, DH), (wo_sb, wot, C)):
            nc.sync.dma_start(
                dst[:].rearrange("p (k d) -> p k d", d=nd),
                src_ap.rearrange("(k p) d -> p k d", p=128))

        v_all = gp.tile([128, (TT // 128) * DH], BF16)   # [128, 8192]
        qT = [gp.tile([128, TT], BF16, tag=f"qT{h}", name=f"qT{h}") for h in range(HL)]
        kT = [gp.tile([128, TT], BF16, tag=f"kT{h}", name=f"kT{h}") for h in range(HL)]

        ident = gp.tile([128, 128], BF16)
        make_identity(nc, ident[:])
        ones_col = gp.tile([128, 1], BF16)
        nc.vector.memset(ones_col[:], 1.0)
        ones_row = gp.tile([1, 128], F32)
        nc.vector.memset(ones_row[:], 1.0)

        # static causal masks for the 4 diagonal block offsets (f32-exact
        # iota, stored bf16 0/1): mask_k keeps [x, y] iff x <= y - 128k
        pairmasks = []
        mtmp = gp.tile([128, 512], F32)
        for m in range(2):
            pm = gp.tile([128, 1024], BF16, tag=f"pmask{m}", name=f"pmask{m}")
            for half in range(2):
                k = 2 * m + half
                nc.vector.memset(mtmp[:], 1.0)
                nc.gpsimd.affine_select(
                    out=mtmp[:], in_=mtmp[:],
                    compare_op=mybir.AluOpType.is_ge, fill=0.0,
                    base=-128 * k, pattern=[[1, 512]], channel_multiplier=-1,
                )
                nc.vector.tensor_copy(pm[:, half * 512:(half + 1) * 512], mtmp[:])
            pairmasks.append(pm)

        # ---- SBUF pools shared by both phases ---------------------------
        xp = ctx.enter_context(tc.tile_pool(name="xin", bufs=20))
        tp = ctx.enter_context(tc.tile_pool(name="trig", bufs=2))
        sp = ctx.enter_context(tc.tile_pool(name="stage", bufs=3))
        rp = ctx.enter_context(tc.tile_pool(name="rtmp", bufs=2))
        ptp = ctx.enter_context(tc.tile_pool(name="ptile", bufs=10))
        atp = ctx.enter_context(tc.tile_pool(name="attnT", bufs=6))
        rdp = ctx.enter_context(tc.tile_pool(name="rden", bufs=2))
        osp = ctx.enter_context(tc.tile_pool(name="ost", bufs=4))

        for _rep in range(repeat):
         if "nop1" in ablate:
             for t_ in (v_all, qT[0], qT[1], kT[0], kT[1]):
                 nc.vector.memset(t_[:], 0.0)
         # ---- phase 1: QKV projections + RoPE + Q/K transposes ----------
         with ExitStack() as p1:
            pqk = p1.enter_context(tc.tile_pool(name="pqk", bufs=4, space="PSUM"))
            pvp = p1.enter_context(tc.tile_pool(name="pv", bufs=2, space="PSUM"))
            pt = p1.enter_context(tc.tile_pool(name="ptr", bufs=2, space="PSUM"))

            for e in (range(NE) if "nop1" not in ablate else []):
                t0 = e * ET  # global row offset of this eighth
                # per-eighth trig tiles doubled for merged q+k RoPE:
                # [128, 2 x (4 x 128)], same data in both halves
                ct2_sb = tp.tile([128, 2 * 4 * 128], BF16, tag="cos")
                st2_sb = tp.tile([128, 2 * 4 * 128], BF16, tag="sin")
                trow = (t0 % T)
                for s_ in range(2):
                    nc.sync.dma_start(
                        ct2_sb[:, s_ * 512:(s_ + 1) * 512].rearrange(
                            "p (tt d) -> p tt d", d=128),
                        cos2[trow:trow + ET, :].rearrange(
                            "(tt p) d -> p tt d", p=128))
                    nc.sync.dma_start(
                        st2_sb[:, s_ * 512:(s_ + 1) * 512].rearrange(
                            "p (tt d) -> p tt d", d=128),
                        sin2[trow:trow + ET, :].rearrange(
                            "(tt p) d -> p tt d", p=128))

                pQK = [pqk.tile([128, 512], F32, tag="pqk", name=f"pQK{_}")
                       for _ in range(4)]

                xcs = []
                for c in range(CT):
                    xc = xp.tile([128, ET], BF16, tag="xc")
                    dma_eng = nc.sync if c % 2 == 0 else nc.scalar
                    dma_eng.dma_start(
                        xc[:], xt[c * 128:(c + 1) * 128, t0:t0 + ET])
                    xcs.append(xc)
                    st = (c == 0)
                    sp_ = (c == CT - 1)
                    for tt in range(4):
                        nc.tensor.matmul(
                            pQK[tt][:], xc[:, tt * 128:(tt + 1) * 128],
                            wqk_sb[:, c * 2 * DH:(c + 1) * 2 * DH],
                            start=st, stop=sp_)

                # V [t, d] straight into v_all (tt-major chains re-reading
                # the resident xcs; 2 PSUM banks suffice)
                for tt in range(4):
                    g = (t0 // 128) + tt
                    pV = pvp.tile([128, 256], F32, tag="pv")
                    for c in range(CT):
                        nc.tensor.matmul(
                            pV[:], xcs[c][:, tt * 128:(tt + 1) * 128],
                            wv_sb[:, c * DH:(c + 1) * DH],
                            start=(c == 0), stop=(c == CT - 1))
                    nc.scalar.copy(v_all[:, g * DH:(g + 1) * DH], pV[:])

                # Q and K staged CONTIGUOUSLY so RoPE runs as 6 big
                # [*,1024] DVE ops instead of 12 [*,512] ones.
                qks = sp.tile([128, 8 * DH], BF16, tag="qks")
                for tt in range(4):
                    nc.vector.tensor_copy(qks[:, tt * DH:(tt + 1) * DH],
                                          pQK[tt][:, 0:256])
                    nc.scalar.copy(qks[:, 4 * DH + tt * DH:4 * DH + (tt + 1) * DH],
                                   pQK[tt][:, 256:512])

                # RoPE in [t, d] layout, half-major head dim (E|O per head):
                #   out_E = E*c - O*s ; out_O = E*s + O*c
                # All operands bf16 + packed 64-wide runs -> DVE fast modes.
                qkr = rp.tile([128, 8 * DH], BF16, tag="qkr")
                tm1 = rp.tile([128, 8 * DH], BF16, tag="tm1")
                tm2 = rp.tile([128, 8 * DH], BF16, tag="tm2")
                cv = ct2_sb[:].rearrange("p (s tt h j) -> p s tt h j",
                                         s=2, tt=4, h=HL)
                sv = st2_sb[:].rearrange("p (s tt h j) -> p s tt h j",
                                         s=2, tt=4, h=HL)
                s6 = qks[:].rearrange(
                    "p (s tt h half j) -> p s tt h half j", s=2, tt=4, h=HL, half=2)
                d6 = qkr[:].rearrange(
                    "p (s tt h half j) -> p s tt h half j", s=2, tt=4, h=HL, half=2)
                t6a = tm1[:].rearrange(
                    "p (s tt h half j) -> p s tt h half j", s=2, tt=4, h=HL, half=2)
                t6b = tm2[:].rearrange(
                    "p (s tt h half j) -> p s tt h half j", s=2, tt=4, h=HL, half=2)
                E, O = s6[:, :, :, :, 0, :], s6[:, :, :, :, 1, :]
                nc.vector.tensor_mul(t6a[:, :, :, :, 0, :], E, cv)
                nc.vector.tensor_mul(t6b[:, :, :, :, 0, :], O, sv)
                nc.vector.tensor_sub(d6[:, :, :, :, 0, :],
                                     t6a[:, :, :, :, 0, :], t6b[:, :, :, :, 0, :])
                nc.vector.tensor_mul(t6a[:, :, :, :, 1, :], E, sv)
                nc.vector.tensor_mul(t6b[:, :, :, :, 1, :], O, cv)
                nc.vector.tensor_add(d6[:, :, :, :, 1, :],
                                     t6a[:, :, :, :, 1, :], t6b[:, :, :, :, 1, :])

                # transpose Q/K blocks [128t, 128d] -> [128d, 128t]
                for s_, dstl in ((0, qT), (1, kT)):
                    for tt in range(4):
                        for h in range(HL):
                            pb = pt.tile([128, 128], BF16, tag="ptr")
                            nc.tensor.transpose(
                                pb[:],
                                qkr[:, s_ * 4 * DH + tt * DH + h * 128:
                                    s_ * 4 * DH + tt * DH + (h + 1) * 128],
                                ident[:])
                            nc.vector.tensor_copy(
                                dstl[h][:, t0 + tt * 128: t0 + (tt + 1) * 128],
                                pb[:])

         if "nop2" in ablate:
             nc.sync.dma_start(out[0:128, :], qT[0][:, 0:C])
             continue
         # ---- phase 2: attention + output projection ---------------------
         with ExitStack() as p2:
             psw = p2.enter_context(tc.tile_pool(name="psw", bufs=2, space="PSUM"))
             pso = p2.enter_context(tc.tile_pool(name="pso", bufs=2, space="PSUM"))
             psa = p2.enter_context(tc.tile_pool(name="psa", bufs=1, space="PSUM"))
             psd = p2.enter_context(tc.tile_pool(name="psd", bufs=1, space="PSUM"))

             for b in range(B):
                 for j in range(T // 512):   # q-chunks of 512 within the batch
                     q0 = b * T + j * 512
                     nkt = 4 * j + 4
                     attnT = []
                     for h in range(HL):
                         pA = psa.tile([128, 512], F32, tag="psa")
                         pDen = psd.tile([1, 512], F32, tag="psd")
                         npair = nkt // 2
                         for p_ in range(npair):
                             diag = (2 * p_ >= 4 * j)
                             pS = psw.tile([128, 1024], F32, tag="psw")
                             ptile = ptp.tile([128, 1024], BF16, tag="ptile")
                             for half in range(2):
                                 i = 2 * p_ + half
                                 g = b * (T // 128) + i
                                 r = i - 4 * j
                                 lo = 128 * r if (diag and r > 0) else 0
                                 nc.tensor.matmul(
                                     pS[:, half * 512 + lo:half * 512 + 512],
                                     kT[h][:, g * 128:(g + 1) * 128],
                                     qT[h][:, q0 + lo:q0 + 512],
                                     start=True, stop=True)
                             if diag:
                                 # per-half exp on the valid range; memset the
                                 # rest so mask-mult stays NaN-free
                                 for half in range(2):
                                     r = 2 * p_ + half - 4 * j
                                     lo = 128 * r
                                     if lo > 0:
                                         nc.vector.memset(
                                             ptile[:, half * 512:half * 512 + lo], 0.0)
                                     nc.scalar.activation(
                                         ptile[:, half * 512 + lo:half * 512 + 512],
                                         pS[:, half * 512 + lo:half * 512 + 512],
                                         mybir.ActivationFunctionType.Exp,
                                         scale=SCALE)
                                 nc.vector.tensor_mul(
                                     ptile[:], ptile[:], pairmasks[p_ - 2 * j][:])
                             else:
                                 nc.scalar.activation(
                                     ptile[:], pS[:],
                                     mybir.ActivationFunctionType.Exp,
                                     scale=SCALE)
                             for half in range(2):
                                 i = 2 * p_ + half
                                 g = b * (T // 128) + i
                                 r = i - 4 * j
                                 lo = 128 * r if (diag and r > 0) else 0
                                 nc.tensor.matmul(
                                     pA[:, lo:512],
                                     v_all[:, g * DH + h * 128: g * DH + (h + 1) * 128],
                                     ptile[:, half * 512 + lo:half * 512 + 512],
                                     start=(i == 0), stop=(i == nkt - 1),
                                     skip_group_check=True)
                                 # denominator straight off each ptile half
                                 # (memset/masked zeros contribute nothing)
                                 nc.tensor.matmul(
                                     pDen[:], ones_col[:],
                                     ptile[:, half * 512:half * 512 + 512],
                                     start=(i == 0), stop=(i == nkt - 1))
                         rden = rdp.tile([1, 512], F32, tag="rden")
                         nc.vector.reciprocal(rden[:], pDen[:])
                         pB = pso.tile([128, 512], F32, tag="pso")
                         nc.tensor.matmul(pB[:], ones_row[:], rden[:],
                                          start=True, stop=True)
                         bc = rdp.tile([128, 512], F32, tag="bc")
                         nc.scalar.copy(bc[:], pB[:])
                         aT = atp.tile([128, 512], BF16, tag=f"aT{h}")
                         nc.vector.tensor_mul(aT[:], pA[:], bc[:])
                         attnT.append(aT)

                     for tt in range(4):
                         r0 = q0 + tt * 128
                         ost = osp.tile([128, C], _odt, tag="ost")
                         for oc in range(4):
                             pO = pso.tile([128, 512], F32, tag="pso")
                             for h in range(HL):
                                 nc.tensor.matmul(
                                     pO[:], attnT[h][:, tt * 128:(tt + 1) * 128],
                                     wo_sb[:, h * C + oc * 512: h * C + oc * 512 + 512],
                                     start=(h == 0), stop=(h == HL - 1))
                             if oc == 0:
                                 nc.scalar.copy(ost[:, 0:512], pO[:])
                             else:
                                 nc.vector.tensor_copy(
                                     ost[:, oc * 512:(oc + 1) * 512], pO[:])
                         if small_out:
                             nc.scalar.dma_start(out[0:128, :], ost[:])
                         else:
                             nc.scalar.dma_start(out[r0:r0 + 128, :], ost[:])

    nc.compile()
    return nc


def _get_nc():
    if "nc" not in _CACHE:
        _CACHE["nc"] = _build()
    return _CACHE["nc"]


# half-major permutation of a head's 128 dims: [0,2,...,126, 1,3,...,127]
_PERM = np.concatenate([np.arange(0, D, 2), np.arange(1, D, 2)])


def _permute_heads(w_slice):
    # w_slice: [DH, C] rows = local head dims (HL heads x 128)
    out = np.empty_like(w_slice)
    for l in range(HL):
        out[l * D:(l + 1) * D] = w_slice[l * D + _PERM]
    return out


def kernel(x, wq, wk, wv, wo, freqs_cos, freqs_sin, mask=None, **_unused):
    bf = ml_dtypes.bfloat16
    nc = _get_nc()

    x = np.asarray(x, dtype=np.float32)
    xt = np.ascontiguousarray(x.reshape(TT, C).T).astype(bf)
    cos2 = np.ascontiguousarray(
        np.tile(np.asarray(freqs_cos, np.float32), (1, HL))).astype(bf)
    sin2 = np.ascontiguousarray(
        np.tile(np.asarray(freqs_sin, np.float32), (1, HL))).astype(bf)

    in_maps = []
    for i in range(NCORES):
        sl = slice(DH * i, DH * (i + 1))
        wq_s = _permute_heads(np.asarray(wq, np.float32)[sl, :])
        wk_s = _permute_heads(np.asarray(wk, np.float32)[sl, :])
        in_maps.append({
            "xt": xt,
            "wqt": np.ascontiguousarray(wq_s.T).astype(bf),
            "wkt": np.ascontiguousarray(wk_s.T).astype(bf),
            "wvt": np.ascontiguousarray(np.asarray(wv, np.float32)[sl, :].T).astype(bf),
            "wot": np.ascontiguousarray(np.asarray(wo, np.float32)[:, sl].T).astype(bf),
            "cos2": cos2,
            "sin2": sin2,
        })

    res = run_bass_kernel_spmd(nc, in_maps, core_ids=list(range(NCORES)))
    acc = np.zeros((TT, C), dtype=np.float32)
    for r in res.results:
        acc += np.asarray(r["out"], dtype=np.float32)
    return acc.reshape(B, T, C)
